# revision 1
# baseline (speedup 1.0000x reference)
"""Trainium2 Bass kernel for nn_BurgersSolver_75333726371954.

Burgers' equation explicit solver: interpolate u0 [64,512] to a 513-point
grid, run the FTCS stencil on [64,512], snapshot every 0.01 time units at
every 2nd spatial point -> [64,257,101].

Strategy (pure data parallel, batch sharded 8 rows/core across 8 cores):
  * Time-step coarsening: the reference runs 5000 steps of dt=1/5000
    (C2 = nu*dt/dx^2 = 0.262).  Diffusion stability allows C2 < 0.5, so
    after a 50-step fine prefix (which reproduces the reference's damping
    of grid-scale noise in the first snapshot window) the remaining
    t in [0.01, 1.0] runs as 2673 steps of dt=1/2700 (C2 = 0.485).
    Offline check vs the 5000-step reference: max rel err 5.0e-4,
    well inside the 2e-2 gate.  2723 total steps vs 5000.
  * Offset-scaled state U~ = C1*u - C2 makes the FTCS update a 3-op
    telescoped form (shifted views of the intermediate are free in the
    access pattern):
        V_i  = (U~_i + 2*C2) * U~_{i-1}          [STT]
        D_i  = V_i - V_{i+1}                     [tensor_sub, shifted view]
        U~_i += D_i                              [tensor_add, in-place]
    which expands to exactly w' = w - (w_r-w_l)*w + C2*(w_l+w_r-2w) for
    w = U~ + C2 (the C2^2 terms cancel in the telescoped difference).
    3 DVE ops/step.  At the fine->coarse switch the state is remapped
    once by the affine U~' = rho*U~ + (rho*C2f - C2c).
  * Layout [128 partitions = 8 batch x 16 spatial chunks of 32,
    free = 32 + 2H ghost columns], H=9.  Ghost zones allow up to H steps
    between partition-crossing halo exchanges; compute range tapers by
    1/side/step.
  * Halo exchange via two DVE stream_shuffle ops (partition rotate +-1
    within each 16-chunk group, bit-exact copy).  No TensorE, no PSUM,
    no cross-engine semaphores on the hot path.
  * Snapshots always land on block boundaries (prefix end; every 3rd
    coarse block = 27 steps = 0.01 t-units): strided 16-col DVE
    tensor-add into an SBUF accumulation area; single DMA out at the
    end; host rescales by 1/C1(phase) and assembles [64,257,101].
  * Writeback-race margins (measured on HW with a probe kernel: a reader
    issued with <~80 cycles of margin after a writer intermittently reads
    stale SBUF; >=~100 cycles is clean): the in-place state update is
    right-padded into the dead tapered ghost columns [hi,W), so every
    consumer of recently-written data trails its writer by >=~90-200
    cycles with no dedicated spacer ops on the hot path.  The residual
    ~90-cycle corner (halo shuffle reading the just-updated core tail)
    can at worst inject a ~1e-3 one-step-stale perturbation, far inside
    the 2e-2 gate (continuous racing measured only 6e-3 total).
  * Output writeout is incremental: every 10 completed snapshots, gpsimd
    DMAs the finished 160-column strip of SN while the solver keeps
    running, leaving only a 16-column tail DMA after the last step.
"""

import numpy as np

# ---- problem constants (hardcoded; must match the reference config) ----
MX = 513
DX = 1.0 / (MX - 1)

FINE_STEPS = 50                  # dt = 1/5000, covers t in [0, 0.01]
COARSE_STEPS = 2673              # dt = 1/2700, covers t in [0.01, 1.0]
DT_F = 1.0 / 5000.0
DT_C = 0.99 / 2673.0             # == 1/2700

C1F = np.float32(DT_F / (2.0 * DX))
C2F = np.float32(0.005 * DT_F / DX ** 2)
LF = np.float32(1.0 - 2.0 * float(C2F))
C1C = np.float32(DT_C / (2.0 * DX))
C2C = np.float32(0.005 * DT_C / DX ** 2)
LC = np.float32(1.0 - 2.0 * float(C2C))
RHO = np.float32(float(C1C) / float(C1F))

NSNAP = 101

NCORES = 8
BPC = 8                          # batch rows per core
NCHUNK = 16                      # spatial chunks per batch row
CH = 32                          # chunk width (NCHUNK*CH == 512)
H = 9                            # max ghost depth == max block length
W = CH + 2 * H                   # tile free width (50)

MASK_UP = [(i // 16) * 16 + ((i % 16) - 1) % 16 for i in range(32)]
MASK_DN = [(i // 16) * 16 + ((i % 16) + 1) % 16 for i in range(32)]


# block plan: (length, C2, L, snapshot_after, rescale_after)
def _block_plan():
    blocks = []
    fine = [9, 9, 9, 9, 9, 5]
    assert sum(fine) == FINE_STEPS
    for i, b in enumerate(fine):
        last = i == len(fine) - 1
        blocks.append([b, C2F, LF, last, last])   # snap + rescale at prefix end
    ncb = COARSE_STEPS // H
    assert ncb * H == COARSE_STEPS
    for i in range(ncb):
        blocks.append([H, C2C, LC, (i + 1) % 3 == 0, False])
    return blocks


def _emit_hotpath(v, ALU, U, T1, T2, S, SN, zbc, blocks, snap_sem=None):
    """Emit the full time loop on the vector engine; returns #snapshots.

    If snap_sem is given, it is incremented whenever the snapshot counter
    reaches a multiple of 10 — the signal for the incremental SN DMA.
    """
    # t=0 snapshot (state is U~ = C1*u - C2; store w = U~ + C2)
    v.tensor_scalar_add(SN[:, 0:16], U[:, H:H + CH:2], float(C2F))
    snap = 1
    for bi, (B, C2, L, do_snap, do_rescale) in enumerate(blocks):
        C2 = float(C2)
        L = float(L)
        for s in range(1, B + 1):
            lo, hi = H - B + s, H + CH + B - s
            # 3-op telescoped FTCS step on offset state  U~ = C1*u - C2:
            #   V_i = (U~_i + 2*C2) * U~_{i-1}
            #   U~'_i = U~_i + V_i - V_{i+1}
            # expands to exactly  w' = w - (w_r-w_l)*w + C2*(w_l+w_r-2w)
            # with w = U~ + C2 (the C2^2 constants cancel in V_i - V_{i+1}).
            # All three ops are right-padded through the dead tapered cols
            # to the tile edge (garbage there is never read before the
            # next halo refill): the write streams keep every
            # 1-op-adjacent reader >=~111 cycles behind its writer
            # (measured clean zone), and uniform full-width ranges chain
            # ~7us/run faster than minimal per-step pads (measured).
            v.scalar_tensor_tensor(T1[:, lo:W], U[:, lo:W],
                                   2.0 * C2, U[:, lo - 1:W - 1],
                                   ALU.add, ALU.mult)
            v.tensor_sub(T2[:, lo:W - 1], T1[:, lo:W - 1], T1[:, lo + 1:W])
            v.tensor_add(U[:, lo:W], U[:, lo:W], T2[:, lo:W])
        if bi + 1 < len(blocks):
            Bn = blocks[bi + 1][0]
            # halo exchange for the next block: ghosts to depth Bn.
            # Margins without explicit spacers: shuffle_L trails the padded
            # update by ~(W-32)+init cycles; the next block's first reader
            # trails shuffle_R by shuffle_L's duration (left ghosts) and by
            # its own 41-cycle stream offset (right ghosts).  These sit at
            # ~90-106 cycles; a rare stale read perturbs by ~1e-3, well
            # inside the error budget.
            v.stream_shuffle(U[:, H - Bn:H],
                             U[:, H + CH - Bn:H + CH], MASK_UP)
            v.stream_shuffle(U[:, H + CH:H + CH + Bn],
                             U[:, H:H + Bn], MASK_DN)
        else:
            # endgame margin before the final snapshot
            v.tensor_sub(S[:, 0:4], T1[:, 0:4], T2[:, 0:4])
            v.tensor_sub(S[:, 4:8], T1[:, 4:8], T2[:, 4:8])
        if do_snap:
            # snapshot stores w = U~ + C2 (host divides by C1); it also
            # doubles as the post-shuffle writeback margin
            ins = v.tensor_scalar_add(SN[:, snap * 16:snap * 16 + 16],
                                      U[:, H:H + CH:2], C2)
            snap += 1
            if snap_sem is not None and snap % 10 == 0:
                ins.then_inc(snap_sem, 1)
        if do_rescale:
            # offset-state phase change: U~' = rho*U~ + (rho*C2F - C2C)
            v.tensor_scalar(U[:], U[:], float(RHO),
                            float(RHO) * float(C2F) - float(C2C),
                            ALU.mult, ALU.add)
            v.tensor_sub(S[:, 0:2], T1[:, 0:2], T2[:, 0:2])
    return snap


_COMPILED = {}


def _build(blocks=None):
    import concourse.bass as bass
    import concourse.mybir as mybir

    F32 = mybir.dt.float32
    ALU = mybir.AluOpType

    nc = bass.Bass()
    x_in = nc.dram_tensor("x", [128, W], F32, kind="ExternalInput")
    y_out = nc.dram_tensor("y", [128, NSNAP * 16], F32, kind="ExternalOutput")

    if blocks is None:
        blocks = _block_plan()

    with (
        nc.semaphore("dma_sem") as dma_sem,
        nc.semaphore("g_sem") as g_sem,
        nc.semaphore("sn_sem") as sn_sem,
        nc.semaphore("v_sem") as v_sem,
        nc.sbuf_tensor("U", [128, W], F32) as U,
        nc.sbuf_tensor("T1", [128, W], F32) as T1,
        nc.sbuf_tensor("T2", [128, W], F32) as T2,
        nc.sbuf_tensor("S", [128, W], F32) as S,
        nc.sbuf_tensor("SN", [128, NSNAP * 16], F32) as SN,
        nc.sbuf_tensor("ZZ", [128, 1], F32) as ZZ,
    ):
        with nc.Block() as block:
            @block.gpsimd
            def _(g):
                # input DMA first so the memsets overlap the transfer
                g.dma_start(U[:], x_in[:]).then_inc(dma_sem, 16)
                g.memset(ZZ[:], 0.0)
                g.memset(T1[:], 0.0)
                g.memset(T2[:], 0.0)
                g.memset(S[:], 0.0).then_inc(g_sem, 1)

            zbc = ZZ[:].to_broadcast([128, 16])

            @block.vector
            def _(v):
                v.wait_ge(dma_sem, 16)
                v.wait_ge(g_sem, 1)
                snap = _emit_hotpath(v, ALU, U, T1, T2, S, SN, zbc, blocks,
                                     snap_sem=sn_sem)
                assert snap <= NSNAP
                v.tensor_copy(S[:, 0:1], ZZ[:]).then_inc(v_sem, 1)

            @block.gpsimd
            def _(g):
                # incremental snapshot writeout, overlapped with compute:
                # chunk j (10 snaps, 160 cols) as soon as it is complete
                for j in range(10):
                    g.wait_ge(sn_sem, j + 1)
                    g.dma_start(y_out[:, j * 160:(j + 1) * 160],
                                SN[:, j * 160:(j + 1) * 160]).then_inc(
                                    dma_sem, 16)
                g.wait_ge(v_sem, 1)
                g.dma_start(y_out[:, 1600:1616],
                            SN[:, 1600:1616]).then_inc(dma_sem, 16)
                g.wait_ge(dma_sem, 16 * 12)

    return nc


def _interp_init(u0):
    """Replicate the reference's 1D border-padded linear interp, f32."""
    u0 = np.asarray(u0, dtype=np.float32)
    n_in = u0.shape[1]
    X = np.linspace(0.0, 1.0, MX, dtype=np.float32)
    pts = X * np.float32(2.0) - np.float32(1.0)
    idx = (pts + np.float32(1.0)) * np.float32(0.5) * np.float32(n_in - 1)
    idx = np.clip(idx, 0.0, np.float32(n_in - 1))
    i0 = np.floor(idx).astype(np.int32)
    i0 = np.clip(i0, 0, n_in - 2)
    frac = (idx - i0.astype(np.float32)).astype(np.float32)
    u0f = u0[:, i0] * (np.float32(1.0) - frac) + u0[:, i0 + 1] * frac
    return u0f[:, :-1].astype(np.float32)   # [B, 512]


def _make_in_maps(u0):
    u_init = _interp_init(u0)                       # [64, 512]
    # offset state U~ = C1*u - C2 (makes the 3-op telescoped step exact)
    w0 = (C1F * u_init - C2F).astype(np.float32)
    cc, xx = np.meshgrid(np.arange(NCHUNK), np.arange(W), indexing="ij")
    src = (cc * CH + xx - H) % 512                  # [16, W]
    in_maps = []
    for core in range(NCORES):
        wrows = w0[core * BPC:(core + 1) * BPC]     # [8, 512]
        tile = wrows[:, src].astype(np.float32)     # [8, 16, W]
        in_maps.append({"x": tile.reshape(128, W)})
    return in_maps


def kernel(u0):
    from concourse.bass_utils import run_bass_kernel_spmd

    u0 = np.asarray(u0, dtype=np.float32)
    B = u0.shape[0]
    assert B == NCORES * BPC and u0.shape[1] == 512

    in_maps = _make_in_maps(u0)

    if "nc" not in _COMPILED:
        _COMPILED["nc"] = _build()
    nc = _COMPILED["nc"]

    res = run_bass_kernel_spmd(nc, in_maps, core_ids=list(range(NCORES)))

    # per-snapshot state scale: snaps 0,1 in C1F units, snaps 2.. in C1C
    inv = np.empty((NSNAP,), dtype=np.float32)
    inv[0:2] = np.float32(1.0) / C1F
    inv[2:] = np.float32(1.0) / C1C

    out = np.empty((B, 257, NSNAP), dtype=np.float32)
    for core in range(NCORES):
        y = res.results[core]["y"]                  # [128, NSNAP*16]
        y = y.reshape(BPC, NCHUNK, NSNAP, 16)       # [b, chunk, t, k]
        u = y * inv[None, None, :, None]
        # spatial index nx = chunk*16 + k  (covers 0..255)
        out[core * BPC:(core + 1) * BPC, 0:256, :] = (
            u.transpose(0, 1, 3, 2).reshape(BPC, 256, NSNAP))
    out[:, 256, :] = out[:, 0, :]
    return out



# revision 5
# speedup vs baseline: 4.5152x; 4.5152x over previous
"""Trainium2 Bass kernel for nn_BurgersSolver_75333726371954.

Burgers' equation explicit solver: interpolate u0 [64,512] to a 513-point
grid, run the FTCS stencil, snapshot every 0.01 time units at every 2nd
spatial point -> [64,257,101].

Strategy (pure data parallel, batch sharded 8 rows/core across 8 cores):
  * Two-grid scheme.  The reference runs 5000 fine steps (dx=1/512,
    dt=1/5000).  The OUTPUT only samples every 2nd spatial point, i.e. it
    lives on a 256-point grid.  So:
      - Phase A (fine prefix): 50 steps with the reference's exact
        discretization covers t in [0, 0.01] (snapshots 0, 1).  This
        reproduces the reference's damping of grid-scale noise, which a
        coarse grid could not (its Nyquist damping factor |1-4*C2| ~ 0.87
        per step would let initial noise survive into snapshot 1).
      - Restriction: by t=0.01 diffusion has annihilated every mode above
        ~1/4 of the fine Nyquist (damping e^{-nu k^2 t} ~ e^-32), so the
        solution is fully representable on the 256-point grid of output
        points.  Restrict by injection (copy even fine points).
      - Phase C (coarse): dx doubles => dx^2 quadruples => dt can be ~4x
        larger at the same diffusion number C2 = nu*dt/dx^2 < 0.5.  Each
        0.01 output interval takes 7 steps (C2 = 0.468) instead of the 27
        a fine grid would need: 99*7 = 693 steps instead of 2673.
    Offline check vs the 5000-step reference: max rel err 3.3e-3 (worst
    at snapshots 2-4, from marginally-resolved modes right after
    restriction), well inside the 2e-2 gate.
  * Offset-scaled state U~ = C1*u - C2 makes the FTCS update a 3-op
    telescoped form (shifted views of the intermediate are free in the
    access pattern):
        V_i  = (U~_i + 2*C2) * U~_{i-1}          [STT]
        D_i  = V_i - V_{i+1}                     [tensor_sub, shifted view]
        U~_i += D_i                              [tensor_add, in-place]
    which expands to exactly w' = w - (w_r-w_l)*w + C2*(w_l+w_r-2w) for
    w = U~ + C2 (the C2^2 terms cancel in the telescoped difference).
    3 DVE ops/step; per the TRN2 cost model each DVE op costs ~60ns SBUF
    access init + ~1.04ns/col, so total time ~ op count, not data size.
    At the fine->coarse restriction the state is remapped once by the
    affine U~c = rho*U~f + (rho*C2F - C2C) fused into the strided copy.
  * Layout: 128 partitions = 8 batch rows x 16 spatial chunks.  Fine tile:
    chunk=32 cols + 2*9 ghosts (W=50).  Coarse tile: chunk=16 cols + 2*7
    ghosts (30 live cols).  Ghost zones allow H steps between
    partition-crossing halo exchanges; compute range tapers by 1/side/step.
    Coarse blocks are exactly 7 steps = 1 snapshot interval, so every
    block boundary is [halo, halo, snapshot].
  * Halo exchange via two DVE stream_shuffle ops (partition rotate +-1
    within each 16-chunk group, bit-exact copy).  No TensorE, no PSUM,
    no cross-engine semaphores on the hot path.
  * Writeback-race margins (measured on HW by the baseline session: a
    reader <~80 DVE cycles behind a writer intermittently reads stale
    SBUF; >=~100 cycles is clean): all coarse hot-path ops are
    right-padded to WC cols so every 1-op-adjacent reader trails its
    writer by ~the full op duration.  The pad columns [30, WC) are never
    refreshed and carry garbage, but contamination propagates only 1
    column per step and blocks are exactly 7 steps, so the taper absorbs
    it structurally and the halo refill rewrites the real ghosts each
    block (same invariant the baseline used for its tapered dead zone).
  * Snapshots: strided 16-col DVE tensor-add into an SBUF accumulation
    area; host rescales by 1/C1(phase) and assembles [64,257,101].
  * Output writeout is incremental: every 10 completed snapshots, gpsimd
    DMAs the finished 160-column strip of SN while the solver keeps
    running, leaving only a 16-column tail DMA after the last step.
"""

import numpy as np

# ---- problem constants (hardcoded; must match the reference config) ----
MX = 513
DX = 1.0 / (MX - 1)

FINE_STEPS = 50                  # dt = 1/5000, covers t in [0, 0.01]
DT_F = 1.0 / 5000.0

C1F = np.float32(DT_F / (2.0 * DX))
C2F = np.float32(0.005 * DT_F / DX ** 2)

NSUB = 7                         # coarse steps per 0.01 output interval
NINT = 99                        # coarse intervals, t in [0.01, 1.0]
DT_C = 0.01 / NSUB
DXC = 2.0 * DX                   # coarse grid spacing (256 points)
C1C = np.float32(DT_C / (2.0 * DXC))
C2C = np.float32(0.005 * DT_C / DXC ** 2)
RHO = np.float32(float(C1C) / float(C1F))
OFF = np.float32(float(RHO) * float(C2F) - float(C2C))

NSNAP = 101

NCORES = 8
BPC = 8                          # batch rows per core
NCHUNK = 16                      # spatial chunks per batch row
CH = 32                          # fine chunk width (NCHUNK*CH == 512)
H = 9                            # fine ghost depth == max fine block length
WF = CH + 2 * H                  # fine tile free width (50)

CH2 = 16                         # coarse chunk width (NCHUNK*CH2 == 256)
H2 = 7                           # coarse ghost depth == coarse block length
WC = 50                          # coarse tile width: 30 live + 20 pad cols.
                                 # Padding sets the writer->reader margin for
                                 # the 1-op-adjacent dependent pairs in the
                                 # step chain (margin ~= op duration ~= 60ns
                                 # + WC*1.04ns).  WC=42 (~100 DVE cycles) was
                                 # measured BAD on HW: systematic stale reads
                                 # turned snap-2 err from 3.3e-3 into 1.2e-1.
                                 # WC=50 matches the baseline's proven-clean
                                 # 108-cycle margin.

FINE_BLOCKS = [9, 9, 9, 9, 9, 5]
assert sum(FINE_BLOCKS) == FINE_STEPS

MASK_UP = [(i // 16) * 16 + ((i % 16) - 1) % 16 for i in range(32)]
MASK_DN = [(i // 16) * 16 + ((i % 16) + 1) % 16 for i in range(32)]


def _emit_hotpath(v, ALU, T, snap_sem=None):
    """Emit the full time loop on the vector engine; returns #snapshots.

    T is a dict of SBUF tensors: U,T1,T2,S (fine, [128,WF]),
    Uc,T1c,T2c (coarse, [128,WC]), SN ([128, NSNAP*16]).
    If snap_sem is given, it is incremented whenever the snapshot counter
    reaches a multiple of 10 — the signal for the incremental SN DMA.
    """
    U, T1, T2, S = T["U"], T["T1"], T["T2"], T["S"]
    Uc, T1c, T2c, SN = T["Uc"], T["T1c"], T["T2c"], T["SN"]

    # t=0 snapshot (state is U~ = C1F*u - C2F; store w = U~ + C2F)
    v.tensor_scalar_add(SN[:, 0:16], U[:, H:H + CH:2], float(C2F))
    snap = 1

    # ---- phase A: fine prefix, exact reference discretization ----
    C2 = float(C2F)
    for bi, B in enumerate(FINE_BLOCKS):
        for s in range(1, B + 1):
            lo = H - B + s
            v.scalar_tensor_tensor(T1[:, lo:WF], U[:, lo:WF],
                                   2.0 * C2, U[:, lo - 1:WF - 1],
                                   ALU.add, ALU.mult)
            v.tensor_sub(T2[:, lo:WF - 1], T1[:, lo:WF - 1], T1[:, lo + 1:WF])
            v.tensor_add(U[:, lo:WF], U[:, lo:WF], T2[:, lo:WF])
        if bi + 1 < len(FINE_BLOCKS):
            Bn = FINE_BLOCKS[bi + 1]
            v.stream_shuffle(U[:, H - Bn:H],
                             U[:, H + CH - Bn:H + CH], MASK_UP)
            v.stream_shuffle(U[:, H + CH:H + CH + Bn],
                             U[:, H:H + Bn], MASK_DN)

    # ---- restriction to the coarse grid (t = 0.01) ----
    # U~c = RHO*U~f + OFF at even fine points.  Op order is margin-driven
    # (a reader <~100 DVE cycles behind a writer reads stale SBUF): snap1
    # separates the restriction write from shuffle_L's read of cols
    # [16:23); two spacers separate shuffle_R's ghost write from the first
    # coarse STT, whose in0 stream reaches the right-ghost cols only ~23
    # cycles in.  Without the spacers the STT deterministically read the
    # PRE-shuffle (memset-zero) ghosts: 1.2e-1 error, measured.
    v.tensor_scalar(Uc[:, H2:H2 + CH2], U[:, H:H + CH:2], float(RHO),
                    float(OFF), ALU.mult, ALU.add)
    v.tensor_scalar_add(SN[:, 16:32], U[:, H:H + CH:2], C2)
    snap = 2
    v.stream_shuffle(Uc[:, 0:H2], Uc[:, CH2:H2 + CH2], MASK_UP)
    v.stream_shuffle(Uc[:, H2 + CH2:H2 + CH2 + H2], Uc[:, H2:2 * H2], MASK_DN)
    v.tensor_sub(S[:, 0:4], T1[:, 0:4], T2[:, 0:4])
    v.tensor_sub(S[:, 4:8], T1[:, 4:8], T2[:, 4:8])

    # ---- phase C: 99 intervals x 7 coarse steps ----
    C2 = float(C2C)
    for k in range(NINT):
        for s in range(1, NSUB + 1):
            lo = s                       # == H2 - NSUB + s
            v.scalar_tensor_tensor(T1c[:, lo:WC], Uc[:, lo:WC],
                                   2.0 * C2, Uc[:, lo - 1:WC - 1],
                                   ALU.add, ALU.mult)
            v.tensor_sub(T2c[:, lo:WC - 1], T1c[:, lo:WC - 1],
                         T1c[:, lo + 1:WC])
            v.tensor_add(Uc[:, lo:WC], Uc[:, lo:WC], T2c[:, lo:WC])
        if k + 1 < NINT:
            # halo exchange for the next block (ghost depth 7)
            v.stream_shuffle(Uc[:, 0:H2], Uc[:, CH2:H2 + CH2], MASK_UP)
            v.stream_shuffle(Uc[:, H2 + CH2:H2 + CH2 + H2],
                             Uc[:, H2:2 * H2], MASK_DN)
        else:
            # endgame margin before the final snapshot
            v.tensor_sub(S[:, 0:4], T1[:, 0:4], T2[:, 0:4])
            v.tensor_sub(S[:, 4:8], T1[:, 4:8], T2[:, 4:8])
        # snapshot stores w = U~c + C2C (host divides by C1C); it also
        # doubles as the post-shuffle writeback margin
        ins = v.tensor_scalar_add(SN[:, snap * 16:snap * 16 + 16],
                                  Uc[:, H2:H2 + CH2], C2)
        snap += 1
        if snap_sem is not None and snap % 10 == 0:
            ins.then_inc(snap_sem, 1)
        # spacer: pads shuffle_R -> next-block-STT ghost-read margin from
        # ~107ns (threshold; one-block-stale ghosts, 4.5e-2 err measured)
        # to ~170ns
        v.tensor_sub(S[:, 0:4], T1[:, 0:4], T2[:, 0:4])
    return snap


_COMPILED = {}


def _build():
    import concourse.bass as bass
    import concourse.mybir as mybir

    F32 = mybir.dt.float32
    ALU = mybir.AluOpType

    nc = bass.Bass()
    x_in = nc.dram_tensor("x", [128, WF], F32, kind="ExternalInput")
    y_out = nc.dram_tensor("y", [128, NSNAP * 16], F32, kind="ExternalOutput")

    with (
        nc.semaphore("dma_sem") as dma_sem,
        nc.semaphore("g_sem") as g_sem,
        nc.semaphore("sn_sem") as sn_sem,
        nc.semaphore("v_sem") as v_sem,
        nc.sbuf_tensor("U", [128, WF], F32) as U,
        nc.sbuf_tensor("T1", [128, WF], F32) as T1,
        nc.sbuf_tensor("T2", [128, WF], F32) as T2,
        nc.sbuf_tensor("S", [128, WF], F32) as S,
        nc.sbuf_tensor("Uc", [128, WC], F32) as Uc,
        nc.sbuf_tensor("T1c", [128, WC], F32) as T1c,
        nc.sbuf_tensor("T2c", [128, WC], F32) as T2c,
        nc.sbuf_tensor("SN", [128, NSNAP * 16], F32) as SN,
        nc.sbuf_tensor("ZZ", [128, 1], F32) as ZZ,
    ):
        tensors = {"U": U, "T1": T1, "T2": T2, "S": S,
                   "Uc": Uc, "T1c": T1c, "T2c": T2c, "SN": SN}
        with nc.Block() as block:
            @block.gpsimd
            def _(g):
                # input DMA first so the memsets overlap the transfer
                g.dma_start(U[:], x_in[:]).then_inc(dma_sem, 16)
                g.memset(ZZ[:], 0.0)
                g.memset(T1[:], 0.0)
                g.memset(T2[:], 0.0)
                g.memset(Uc[:], 0.0)
                g.memset(T1c[:], 0.0)
                g.memset(T2c[:], 0.0)
                g.memset(S[:], 0.0).then_inc(g_sem, 1)

            @block.vector
            def _(v):
                v.wait_ge(dma_sem, 16)
                v.wait_ge(g_sem, 1)
                snap = _emit_hotpath(v, ALU, tensors, snap_sem=sn_sem)
                assert snap == NSNAP
                v.tensor_copy(S[:, 0:1], ZZ[:]).then_inc(v_sem, 1)

            @block.gpsimd
            def _(g):
                # incremental snapshot writeout, overlapped with compute:
                # chunk j (10 snaps, 160 cols) as soon as it is complete
                for j in range(10):
                    g.wait_ge(sn_sem, j + 1)
                    g.dma_start(y_out[:, j * 160:(j + 1) * 160],
                                SN[:, j * 160:(j + 1) * 160]).then_inc(
                                    dma_sem, 16)
                g.wait_ge(v_sem, 1)
                g.dma_start(y_out[:, 1600:1616],
                            SN[:, 1600:1616]).then_inc(dma_sem, 16)
                g.wait_ge(dma_sem, 16 * 12)

    return nc


def _interp_init(u0):
    """Replicate the reference's 1D border-padded linear interp, f32."""
    u0 = np.asarray(u0, dtype=np.float32)
    n_in = u0.shape[1]
    X = np.linspace(0.0, 1.0, MX, dtype=np.float32)
    pts = X * np.float32(2.0) - np.float32(1.0)
    idx = (pts + np.float32(1.0)) * np.float32(0.5) * np.float32(n_in - 1)
    idx = np.clip(idx, 0.0, np.float32(n_in - 1))
    i0 = np.floor(idx).astype(np.int32)
    i0 = np.clip(i0, 0, n_in - 2)
    frac = (idx - i0.astype(np.float32)).astype(np.float32)
    u0f = u0[:, i0] * (np.float32(1.0) - frac) + u0[:, i0 + 1] * frac
    return u0f[:, :-1].astype(np.float32)   # [B, 512]


def _make_in_maps(u0):
    u_init = _interp_init(u0)                       # [64, 512]
    # offset state U~ = C1F*u - C2F (makes the 3-op telescoped step exact)
    w0 = (C1F * u_init - C2F).astype(np.float32)
    cc, xx = np.meshgrid(np.arange(NCHUNK), np.arange(WF), indexing="ij")
    src = (cc * CH + xx - H) % 512                  # [16, WF]
    in_maps = []
    for core in range(NCORES):
        wrows = w0[core * BPC:(core + 1) * BPC]     # [8, 512]
        tile = wrows[:, src].astype(np.float32)     # [8, 16, WF]
        in_maps.append({"x": tile.reshape(128, WF)})
    return in_maps


def kernel(u0):
    from concourse.bass_utils import run_bass_kernel_spmd

    u0 = np.asarray(u0, dtype=np.float32)
    B = u0.shape[0]
    assert B == NCORES * BPC and u0.shape[1] == 512

    in_maps = _make_in_maps(u0)

    if "nc" not in _COMPILED:
        _COMPILED["nc"] = _build()
    nc = _COMPILED["nc"]

    res = run_bass_kernel_spmd(nc, in_maps, core_ids=list(range(NCORES)))

    # per-snapshot state scale: snaps 0,1 in C1F units, snaps 2.. in C1C
    inv = np.empty((NSNAP,), dtype=np.float32)
    inv[0:2] = np.float32(1.0) / C1F
    inv[2:] = np.float32(1.0) / C1C

    out = np.empty((B, 257, NSNAP), dtype=np.float32)
    for core in range(NCORES):
        y = res.results[core]["y"]                  # [128, NSNAP*16]
        y = y.reshape(BPC, NCHUNK, NSNAP, 16)       # [b, chunk, t, k]
        u = y * inv[None, None, :, None]
        # spatial index nx = chunk*16 + k  (covers 0..255)
        out[core * BPC:(core + 1) * BPC, 0:256, :] = (
            u.transpose(0, 1, 3, 2).reshape(BPC, 256, NSNAP))
    out[:, 256, :] = out[:, 0, :]
    return out
